# revision 1
# baseline (speedup 1.0000x reference)
"""Trainium2 Bass kernel: out = IFFT(D .* FFT(A .* x)) over dim 1 (4096),
batch 8192, A/D scale re/im independently. Data-parallel over 8 NeuronCores.

4-step FFT factorization (4096 = 64 x 64): each FFT = two DFT-64 stages as
stacked-real matmuls (float32r), twiddle multiplies on DVE/Pool, f32r PE
corner-turn transposes between stages. Raw Bass engine blocks with explicit
counting semaphores; skewed software pipeline (build_nc_pipe) so all five
engines overlap across 8-row groups.
"""
import sys

for p in ("/opt/trn_rl_repo",):
    if p not in sys.path:
        sys.path.insert(0, p)

import numpy as np
from concourse import bass, mybir
from concourse.bass_utils import run_bass_kernel_spmd

F32 = mybir.dt.float32
F32R = mybir.dt.float32r
F16 = mybir.dt.float16
GROUP = 8
MUL = mybir.AluOpType.mult
ADD = mybir.AluOpType.add


import numpy as np

R64 = 64
N = 4096


def _dft():
    a = np.arange(R64)
    F = np.exp(-2j * np.pi * np.outer(a, a) / R64)
    return F


def make_mats():
    """Stationary matrices, concatenated [128, 8*128] f32."""
    F = _dft()
    Fr, Fi = F.real, F.imag
    G = np.conj(F) / R64
    Gr, Gi = G.real, G.imag

    def blkdiag(M):
        out = np.zeros((128, 128))
        out[:64, :64] = M
        out[64:, 64:] = M
        return out

    cM1_frT = blkdiag(Fr.T)
    cM1_nfiT = blkdiag(-Fi.T)
    cM1_fiT = blkdiag(Fi.T)
    cM3 = np.block([[Gr.T, Gi.T], [-Gi.T, Gr.T]])
    cM3sw = np.block([[Gi.T, Gr.T], [Gr.T, -Gi.T]])
    cM2a = np.zeros((128, 128))
    cM2a[:64] = np.concatenate([Fr.T, Fi.T], axis=1)
    cM2b = np.zeros((128, 128))
    cM2b[:64] = np.concatenate([-Fi.T, Fr.T], axis=1)
    I128 = np.eye(128)
    cM4a = np.zeros((128, 128))
    cM4a[:64] = np.concatenate([Gr.T, Gi.T], axis=1)
    cM4b = np.zeros((128, 128))
    cM4b[:64] = np.concatenate([-Gi.T, Gr.T], axis=1)
    mats = np.concatenate(
        [cM1_frT, cM1_nfiT, cM1_fiT, cM3, cM3sw, cM2a, cM2b, I128, cM4a, cM4b], axis=1
    )
    return mats.astype(np.float32)


def make_coefs(A, D):
    """Elementwise coefficient tiles, concatenated [128, 6*512] f32.

    A, D: (1, 4096, 2) f32.
    order: cA2, cTr, cT1i, cT1i_neg, cT2c, cD2
    """
    k1 = np.arange(R64)[:, None]
    n2 = np.arange(R64)[None, :]
    ang = 2 * np.pi * (k1 * n2) / N
    T1r = np.cos(ang)
    T1i = -np.sin(ang)  # T1 = exp(-i ang)
    # T2 = conj(w)^(m1*k1): T2r = T1r, T2i = +sin = -T1i

    # cA2 slot = [cA2re | cA2im], each [128 x 256] = [n1-pairstack x (pr n2)]
    A2 = A.reshape(64, 64, 2)  # [n1][n2][ri]
    cA2re = np.tile(np.vstack([A2[:, :, 0], A2[:, :, 0]]), (1, 4))
    cA2im = np.tile(np.vstack([A2[:, :, 1], A2[:, :, 1]]), (1, 4))
    cA2 = np.concatenate([cA2re, cA2im], axis=1)  # [128 x 512]

    def stk8(top, bot):
        return np.tile(np.vstack([top, bot]), (1, 8))

    cTr = stk8(T1r, T1r)
    # cT1i slot = mixed: first 256 cols = [T1i;T1i]-rep (for t3 on P1),
    # second 256 = [-T1i;-T1i]-rep (for t2 on P2)
    stk4 = lambda t: np.tile(np.vstack([t, t]), (1, 4))
    cT1i = np.concatenate([stk4(T1i), stk4(-T1i)], axis=1)
    cT1i_neg = stk8(-T1i, -T1i)  # unused (layout compat)
    # T2-C coeff applied to psWsw=[Wi;Wr]: [-T2i ; +T2i] = [T1i ; -T1i]
    cT2c = stk8(T1i, -T1i)
    D2 = D.reshape(4096, 2).reshape(64, 64, 2)  # [k2][k1][ri]
    cD2 = stk8(D2[:, :, 0], D2[:, :, 1])
    coefs = np.concatenate([cA2, cTr, cT1i, cT1i_neg, cT2c, cD2], axis=1)
    return coefs.astype(np.float32)


def emulate_group(xg, mats, coefs):
    """Emulate one 8-row group with the exact tile ops.

    xg: (8, 4096, 2) f32. Returns (8, 4096, 2) f32.
    """
    m = {}
    names = ["cM1_frT", "cM1_nfiT", "cM1_fiT", "cM3", "cM3sw", "cM2a", "cM2b", "I128", "cM4a", "cM4b"]
    for i, nm in enumerate(names):
        m[nm] = mats[:, i * 128 : (i + 1) * 128]
    c = {}
    cn = ["cA2", "cTr", "cT1i", "cT1i_neg", "cT2c", "cD2"]
    for i, nm in enumerate(cn):
        c[nm] = coefs[:, i * 512 : (i + 1) * 512]

    # DMA in: u2 [128 x (pr, n2, ri)] per pair h-stacked
    x4 = xg.reshape(8, 64, 64, 2)
    u2 = np.zeros((128, 4, 64, 2), np.float32)
    for pr in range(4):
        u2[:64, pr] = x4[2 * pr]
        u2[64:, pr] = x4[2 * pr + 1]
    re_cols = u2[:, :, :, 0].reshape(128, 256) * c["cA2"][:, 0:256]
    im_cols = u2[:, :, :, 1].reshape(128, 256) * c["cA2"][:, 256:512]
    psP1 = m["cM1_frT"].T @ re_cols + m["cM1_nfiT"].T @ im_cols
    psP2 = m["cM1_fiT"].T @ re_cols + m["cM1_frT"].T @ im_cols
    ps12 = np.concatenate([psP1, psP2], axis=1)  # [128 x 512]
    t14 = ps12 * c["cTr"]          # [t1 | t4]
    t23 = ps12 * c["cT1i"]         # [t3 | t2]
    qr = t14[:, 0:256] + t23[:, 256:512]
    qi = t23[:, 0:256] + t14[:, 256:512]
    psT_a = np.zeros((64, 512), np.float32)
    psT_b = np.zeros((64, 512), np.float32)
    for pr in range(4):
        psT_a[:, 128 * pr : 128 * pr + 128] = qr[:, 64 * pr : 64 * pr + 64].T
        psT_b[:, 128 * pr : 128 * pr + 128] = qi[:, 64 * pr : 64 * pr + 64].T
    psZ = m["cM2a"][0:64].T @ psT_a + m["cM2b"][0:64].T @ psT_b
    zs = psZ * c["cD2"]
    psW = m["cM3"].T @ zs
    psWsw = m["cM3sw"].T @ zs
    s_ = psW * c["cTr"]
    c_ = psWsw * c["cT2c"]
    wp = s_ + c_
    psWt_a = np.zeros((64, 512), np.float32)
    psWt_b = np.zeros((64, 512), np.float32)
    for r in range(8):
        psWt_a[:, 64 * r : 64 * r + 64] = wp[0:64, 64 * r : 64 * r + 64].T
        psWt_b[:, 64 * r : 64 * r + 64] = wp[64:128, 64 * r : 64 * r + 64].T
    psO = m["cM4a"][0:64].T @ psWt_a + m["cM4b"][0:64].T @ psWt_b
    outt = np.zeros((64, 8, 64, 2), np.float32)
    psO4 = psO.reshape(128, 8, 64)
    outt[:, :, :, 0] = psO4[0:64]
    outt[:, :, :, 1] = psO4[64:128]
    # outt [m2 x (r, m1, ri)] -> per row [m2,64*2] = row (4096,2)
    out = outt.transpose(1, 0, 2, 3).reshape(8, 4096, 2)
    return out




class Counts:
    """Cumulative instruction counters per semaphore."""

    def __init__(self):
        self.c = {"pe": 0, "dve": 0, "act": 0, "pool": 0,
                  "in0": 0, "in1": 0, "out0": 0, "out1": 0, "cdma": 0}

    def inc(self, k, n=1):
        self.c[k] += n
        return self.c[k]


def build_nc(R, nrep=1):
    assert R % GROUP == 0
    ngroups_data = R // GROUP
    ngroups = ngroups_data * nrep
    nc = bass.Bass("TRN2", target_bir_lowering=False)

    x_ext = nc.declare_dram_parameter("x", [R, 64, 64, 2], F32, isOutput=False)
    mats_ext = nc.declare_dram_parameter("cmats_d", [128, 1280], F32, isOutput=False)
    coef_ext = nc.declare_dram_parameter("ccoef_d", [128, 3072], F32, isOutput=False)
    out_ext = nc.declare_dram_parameter("out", [R, 64, 128], F32, isOutput=True)

    from contextlib import ExitStack

    ctx = ExitStack()
    sb = {}

    def sbuf(name, shape, dt):
        sb[name] = ctx.enter_context(nc.sbuf_tensor(name, shape, dt))
        return sb[name]

    def psum(name, shape, dt):
        sb[name] = ctx.enter_context(nc.psum_tensor(name, shape, dt))
        return sb[name]

    # constants
    cmats_f = sbuf("cmats_f", [128, 1280], F32)
    ccoef = sbuf("ccoef", [128, 3072], F32)
    matr = [sbuf(f"matr{i}", [128, 128], F32R) for i in range(10)]
    i128h = sbuf("i128h", [128, 128], F16)
    # work tiles, double buffered
    for p in range(2):
        sbuf(f"u2_{p}", [128, 4, 64, 2], F32)
        sbuf(f"u2re_{p}", [128, 4, 64], F32R)
        sbuf(f"u2im_{p}", [128, 4, 64], F32R)
        sbuf(f"t14_{p}", [128, 512], F32)
        sbuf(f"t23_{p}", [128, 512], F32)
        sbuf(f"qr_{p}", [128, 256], F32R)
        sbuf(f"qi_{p}", [128, 256], F32R)
        sbuf(f"vtab_{p}", [64, 1024], F32R)
        sbuf(f"zs_{p}", [128, 8, 64], F32R)
        sbuf(f"s_{p}", [128, 512], F32)
        sbuf(f"c_{p}", [128, 512], F32)
        sbuf(f"wp_{p}", [128, 512], F32R)
        sbuf(f"wtab_{p}", [64, 1024], F32R)
        sbuf(f"outt_{p}", [64, 8, 64, 2], F32)
    psum("ps12", [128, 512], F32)
    psum("psT", [64, 1024], F32R)
    psum("psZO", [128, 8, 64], F32)
    psum("psW", [128, 512], F32)
    psum("psWsw", [128, 512], F32)
    psum("psWt", [64, 1024], F32R)

    cn = Counts()
    # emission lists: (kind, payload...) executed inside engine bodies later.
    prog = {k: [] for k in ("sp", "pe", "dve", "act", "pool")}

    DMA_KEYS = ("in0", "in1", "out0", "out1", "cdma")

    def emit(eng, fn, sem_key):
        prog[eng].append(("op", fn, sem_key))
        return cn.inc(sem_key, 16 if sem_key in DMA_KEYS else 1)

    def wait(eng, sem_key, val):
        if val > 0:
            prog[eng].append(("wait", sem_key, val))

    def coef(i):
        return ccoef[:, 512 * i : 512 * i + 512]

    # ---------------- constants setup ----------------
    emit("sp", lambda e: e.dma_start(out=cmats_f[:, :], in_=mats_ext[:, :]), "cdma")
    d_all = emit("sp", lambda e: e.dma_start(out=ccoef[:, :], in_=coef_ext[:, :]), "cdma")
    wait("act", "cdma", d_all)
    conv_done = 0
    for i in range(10):
        def cp(e, i=i):
            return e.copy(matr[i][:, :], cmats_f[:, 128 * i : 128 * i + 128])
        conv_done = emit("act", cp, "act")
    (cM1_frT, cM1_nfiT, cM1_fiT, cM3, cM3sw, cM2a, cM2b, _I128r, cM4a, cM4b) = matr
    I128 = _I128r  # f32r identity for f32r transposes
    cA2, cTr, cT1i, cT1i_neg, cT2c, cD2 = [coef(i) for i in range(6)]

    # per-group event snapshots for cross-group WAR waits
    ev = {}

    # prologue waits for engines that read constants
    wait("dve", "cdma", d_all)
    wait("pool", "cdma", d_all)
    wait("pe", "act", conv_done)

    for g in range(ngroups):
        gd = g % ngroups_data  # data index (repeats re-process same rows)
        p = g % 2
        u2 = sb[f"u2_{p}"]
        u2re, u2im = sb[f"u2re_{p}"], sb[f"u2im_{p}"]
        t14, t23 = sb[f"t14_{p}"], sb[f"t23_{p}"]
        qr, qi = sb[f"qr_{p}"], sb[f"qi_{p}"]
        vtab, zs = sb[f"vtab_{p}"], sb[f"zs_{p}"]
        s_, c_, wp = sb[f"s_{p}"], sb[f"c_{p}"], sb[f"wp_{p}"]
        wtab, outt = sb[f"wtab_{p}"], sb[f"outt_{p}"]
        ps12 = sb["ps12"]
        psP1 = ps12[:, 0:256]
        psP2 = ps12[:, 256:512]
        psT = sb["psT"]
        psZO, psW, psWsw = sb["psZO"], sb["psW"], sb["psWsw"]
        psWt = sb["psWt"]

        # ---- SP: input DMAs for group g (u2[p] free after A-scale g-2) ----
        if g >= 2:
            wait("sp", "pool", ev[g - 2, "Adone"])
        inkey = f"in{p}"
        in_done = 0
        for pr in range(4):
            def din(e, gd=gd, pr=pr, u2=u2):
                xpr = x_ext[
                    8 * gd + 2 * pr : 8 * gd + 2 * pr + 2, :, :, :
                ].rearrange("h n1 n2 ri -> (h n1) n2 ri")
                return e.dma_start(out=u2[:, pr, :, :], in_=xpr)
            in_done = emit("sp", din, inkey)
        ev[g, "dma_in"] = in_done

        # ---- Pool: A-scale (waits per-pair DMA; u2r WAR: M1 reads g-2) ----
        if g >= 2:
            wait("pool", "pe", ev[g - 2, "M1done"])
        a_end = 0
        wait("pool", inkey, in_done)
        for pr in range(4):
            def asc_re(e, pr=pr, u2=u2, u2re=u2re):
                return e.tensor_tensor(
                    u2re[:, pr, :], u2[:, pr, :, 0],
                    cA2[:, 64 * pr : 64 * pr + 64], MUL,
                )
            def asc_im(e, pr=pr, u2=u2, u2im=u2im):
                return e.tensor_tensor(
                    u2im[:, pr, :], u2[:, pr, :, 1],
                    cA2[:, 256 + 64 * pr : 256 + 64 * pr + 64], MUL,
                )
            emit("pool", asc_re, "pool")
            a_end = emit("pool", asc_im, "pool")
        ev[g, "Adone"] = a_end

        # ---- PE: M1 (psP1/psP2 free after t-ops of g-1) ----
        if g >= 1:
            wait("pe", "dve", ev[g - 1, "Tdone"])
        wait("pe", "pool", a_end)
        re_c = u2re[:, :, :]
        im_c = u2im[:, :, :]
        emit("pe", lambda e, r=re_c: e.matmul(psP1, cM1_frT[:, :], r, start=True, stop=False), "pe")
        emit("pe", lambda e, i=im_c: e.matmul(psP1, cM1_nfiT[:, :], i, start=False, stop=False), "pe")
        emit("pe", lambda e, r=re_c: e.matmul(psP2, cM1_fiT[:, :], r, start=False, stop=False), "pe")
        m1_end = emit("pe", lambda e, i=im_c: e.matmul(psP2, cM1_frT[:, :], i, start=False, stop=True), "pe")
        ev[g, "M1done"] = m1_end

        # ---- DVE: T1 products (t tiles WAR: pool adds g-1) ----
        if g >= 1:
            wait("dve", "pool", ev[g - 1, "QDone"])
        wait("dve", "pe", m1_end)
        emit("dve", lambda e, t14=t14: e.tensor_tensor(t14[:, :], ps12[:, :], cTr, MUL), "dve")
        t_end = emit("dve", lambda e, t23=t23: e.tensor_tensor(t23[:, :], ps12[:, :], cT1i, MUL), "dve")
        ev[g, "Tdone"] = t_end

        # ---- Pool: qr/qi adds (qr/qi WAR: Tp1 of g-1) ----
        if g >= 1:
            wait("pool", "pe", ev[g - 1, "Tp1done"])
        wait("pool", "dve", t_end)
        emit("pool", lambda e, qr=qr, t14=t14, t23=t23: e.tensor_tensor(
            qr[:, :], t14[:, 0:256], t23[:, 256:512], ADD), "pool")
        q_end = emit("pool", lambda e, qi=qi, t14=t14, t23=t23: e.tensor_tensor(
            qi[:, :], t23[:, 0:256], t14[:, 256:512], ADD), "pool")
        ev[g, "QDone"] = q_end

        # ---- PE: Tp1 (psT free after vt-evict g-1) ----
        if g >= 1:
            wait("pe", "act", ev[g - 1, "vtdone"])
        wait("pe", "pool", q_end)
        tp1_end = 0
        for pr in range(4):
            def tpa(e, pr=pr, qr=qr):
                return e.matmul(
                    psT[:, 128 * pr : 128 * pr + 128], qr[:, 64 * pr : 64 * pr + 64],
                    I128[:, :], is_transpose=True, start=(pr == 0), stop=(pr == 3),
                )
            def tpb(e, pr=pr, qi=qi):
                return e.matmul(
                    psT[:, 512 + 128 * pr : 512 + 128 * pr + 128], qi[:, 64 * pr : 64 * pr + 64],
                    I128[:, :], is_transpose=True, start=(pr == 0), stop=(pr == 3),
                )
            emit("pe", tpa, "pe")
            tp1_end = emit("pe", tpb, "pe")
        ev[g, "Tp1done"] = tp1_end

        # ---- ACT: vt evict (vt WAR: M2 of g-1) ----
        if g >= 1:
            wait("act", "pe", ev[g - 1, "M2done"])
        wait("act", "pe", tp1_end)
        vt_end = emit("act", lambda e, vtab=vtab: e.copy(vtab[:, :], psT[:, :]), "act")
        ev[g, "vtdone"] = vt_end

        # ---- PE: M2 (psZO free after zs g-1 AND outt evicts g-1) ----
        if g >= 1:
            wait("pe", "dve", ev[g - 1, "zsdone"])
            wait("pe", "act", ev[g - 1, "outtdone"])
        wait("pe", "act", vt_end)
        emit("pe", lambda e, vtab=vtab: e.matmul(psZO[:, :, :], cM2a[0:64, :], vtab[:, 0:512], start=True, stop=False), "pe")
        m2_end = emit("pe", lambda e, vtab=vtab: e.matmul(psZO[:, :, :], cM2b[0:64, :], vtab[:, 512:1024], start=False, stop=True), "pe")
        ev[g, "M2done"] = m2_end

        # ---- DVE: D-scale (zs WAR: M3/M3sw of g-1) ----
        if g >= 1:
            wait("dve", "pe", ev[g - 1, "M3done"])
        wait("dve", "pe", m2_end)
        cD2v = cD2.rearrange("p (r k) -> p r k", k=64)
        zs_end = emit("dve", lambda e, zs=zs, cD2v=cD2v: e.tensor_tensor(zs[:, :, :], psZO[:, :, :], cD2v, MUL), "dve")
        ev[g, "zsdone"] = zs_end

        # ---- PE: M3 + M3sw (psW free after s_ g-1; psWsw after c_ g-1) ----
        if g >= 1:
            wait("pe", "dve", ev[g - 1, "sdone"])
            wait("pe", "dve", ev[g - 1, "wswdone"])
        wait("pe", "dve", zs_end)
        emit("pe", lambda e, zs=zs: e.matmul(psW[:, :], cM3[:, :], zs[:, :, :], start=True, stop=True), "pe")
        m3_end = emit("pe", lambda e, zs=zs: e.matmul(psWsw[:, :], cM3sw[:, :], zs[:, :, :], start=True, stop=True), "pe")
        ev[g, "M3done"] = m3_end

        # ---- DVE: s_ = T2r*W (s_ WAR: wp-add g-1) ----
        if g >= 1:
            wait("dve", "pool", ev[g - 1, "wpdone"])
        wait("dve", "pe", m3_end)
        s_end = emit("dve", lambda e, s_=s_: e.tensor_tensor(s_[:, :], psW[:, :], cTr, MUL), "dve")
        ev[g, "sdone"] = s_end

        # ---- DVE: c_ = cT2c * psWsw (direct psum read) ----
        c_end = emit("dve", lambda e, c_=c_: e.tensor_tensor(c_[:, :], psWsw[:, :], cT2c, MUL), "dve")
        ev[g, "wswdone"] = c_end  # psWsw free after c_

        # ---- Pool: wp = s_ + c_ ----
        if g >= 1:
            wait("pool", "pe", ev[g - 1, "Tp2done"])
        wait("pool", "dve", c_end)
        wp_end = emit("pool", lambda e, wp=wp, s_=s_, c_=c_: e.tensor_tensor(wp[:, :], s_[:, :], c_[:, :], ADD), "pool")
        ev[g, "wpdone"] = wp_end

        # ---- PE: Tp2 (psWt free after wt-evict g-1) ----
        if g >= 1:
            wait("pe", "act", ev[g - 1, "wtdone"])
        wait("pe", "pool", wp_end)
        tp2_end = 0
        for r in range(8):
            def tpc(e, r=r, wp=wp):
                return e.matmul(
                    psWt[:, 64 * r : 64 * r + 64], wp[0:64, 64 * r : 64 * r + 64],
                    I128[0:64, 0:64], is_transpose=True, start=(r == 0), stop=(r == 7),
                )
            def tpd(e, r=r, wp=wp):
                return e.matmul(
                    psWt[:, 512 + 64 * r : 512 + 64 * r + 64], wp[64:128, 64 * r : 64 * r + 64],
                    I128[64:128, 64:128], is_transpose=True, start=(r == 0), stop=(r == 7),
                )
            emit("pe", tpc, "pe")
            tp2_end = emit("pe", tpd, "pe")
        ev[g, "Tp2done"] = tp2_end

        # ---- ACT: wt evict (wt WAR: M4 g-1) ----
        if g >= 1:
            wait("act", "pe", ev[g - 1, "M4done"])
        wait("act", "pe", tp2_end)
        wt_end = emit("act", lambda e, wtab=wtab: e.copy(wtab[:, :], psWt[:, :]), "act")
        ev[g, "wtdone"] = wt_end

        # ---- PE: M4 (psO free after outt evicts g-1) ----
        if g >= 1:
            wait("pe", "act", ev[g - 1, "outtdone"])
        wait("pe", "act", wt_end)
        emit("pe", lambda e, wtab=wtab: e.matmul(psZO[:, :, :], cM4a[0:64, :], wtab[:, 0:512], start=True, stop=False), "pe")
        m4_end = emit("pe", lambda e, wtab=wtab: e.matmul(psZO[:, :, :], cM4b[0:64, :], wtab[:, 512:1024], start=False, stop=True), "pe")
        ev[g, "M4done"] = m4_end

        # ---- ACT: outt interleave evicts (outt WAR: out-dma g-2) ----
        if g >= 2:
            wait("act", f"out{p}", ev[g - 2, "dma_out"])
        wait("act", "pe", m4_end)
        emit("act", lambda e, outt=outt: e.copy(outt[:, :, :, 0], psZO[0:64, :, :]), "act")
        o_end = emit("act", lambda e, outt=outt: e.copy(outt[:, :, :, 1], psZO[64:128, :, :]), "act")
        ev[g, "outtdone"] = o_end

        # ---- SP: output DMA ----
        wait("sp", "act", o_end)
        def dout(e, gd=gd, outt=outt):
            og = out_ext[8 * gd : 8 * gd + 8, :, :].rearrange("r m2 f -> m2 r f")
            return e.dma_start(
                out=og, in_=outt[:, :, :, :].rearrange("p r m1 ri -> p r (m1 ri)")
            )
        ev[g, "dma_out"] = emit("sp", dout, f"out{p}")

    # final: SP waits for all DMA completions
    for k in ("in0", "in1", "out0", "out1"):
        wait("sp", k, cn.c[k])

    # ---------------- lower to engine blocks ----------------
    with (
        nc.Block() as block,
        nc.semaphore("pe_sem") as pe_sem,
        nc.semaphore("dve_sem") as dve_sem,
        nc.semaphore("act_sem") as act_sem,
        nc.semaphore("pool_sem") as pool_sem,
        nc.semaphore("in0_sem") as in0_sem,
        nc.semaphore("in1_sem") as in1_sem,
        nc.semaphore("out0_sem") as out0_sem,
        nc.semaphore("out1_sem") as out1_sem,
        nc.semaphore("cdma_sem") as cdma_sem,
    ):
        sems = {"pe": pe_sem, "dve": dve_sem, "act": act_sem, "pool": pool_sem,
                "in0": in0_sem, "in1": in1_sem, "out0": out0_sem, "out1": out1_sem,
                "cdma": cdma_sem}

        def runner(eng_name):
            def body(e):
                for item in prog[eng_name]:
                    if item[0] == "drain":
                        e.drain()
                    elif item[0] == "wait":
                        e.wait_ge(sems[item[1]], item[2])
                    else:
                        _, fn, sem_key = item
                        inst = fn(e)
                        inst.then_inc(sems[sem_key], 16 if sem_key in DMA_KEYS else 1)
            return body

        block.sync(runner("sp"))
        block.tensor(runner("pe"))
        block.vector(runner("dve"))
        block.scalar(runner("act"))
        block.gpsimd(runner("pool"))

    ctx.close()
    return nc




def build_nc_pipe(R, nrep=1):
    """Skewed software pipeline: each tick k, every engine works on staggered
    group indices so cross-group overlap is maximal. PSUM couplings (producer
    and consumer of one bank in the same tick): {M1->t, M4->outt} on ps12
    (aliased), {Tp1->vt} psT, {M2->zs} psZ, {M3/M3sw->s_/c_} psW/psWsw,
    {Tp2->wt} psWt. All other handoffs have >=1 tick of slack."""
    assert R % GROUP == 0
    ngroups_data = R // GROUP
    ngroups = ngroups_data * nrep
    nc = bass.Bass("TRN2", target_bir_lowering=False)

    x_ext = nc.declare_dram_parameter("x", [R, 64, 64, 2], F32, isOutput=False)
    mats_ext = nc.declare_dram_parameter("cmats_d", [128, 1280], F32, isOutput=False)
    coef_ext = nc.declare_dram_parameter("ccoef_d", [128, 3072], F32, isOutput=False)
    out_ext = nc.declare_dram_parameter("out", [R, 64, 128], F32, isOutput=True)

    from contextlib import ExitStack

    ctx = ExitStack()
    sb = {}

    def sbuf(name, shape, dt):
        sb[name] = ctx.enter_context(nc.sbuf_tensor(name, shape, dt))
        return sb[name]

    def psum(name, shape, dt):
        sb[name] = ctx.enter_context(nc.psum_tensor(name, shape, dt))
        return sb[name]

    cmats_f = sbuf("cmats_f", [128, 1280], F32)
    ccoef = sbuf("ccoef", [128, 3072], F32)
    matr = [sbuf(f"matr{i}", [128, 128], F32R) for i in range(10)]
    for p in range(2):
        sbuf(f"u2_{p}", [128, 4, 64, 2], F32)
        sbuf(f"u2re_{p}", [128, 4, 64], F32R)
        sbuf(f"u2im_{p}", [128, 4, 64], F32R)
        sbuf(f"t14_{p}", [128, 512], F32)
        sbuf(f"t23_{p}", [128, 512], F32)
        sbuf(f"qr_{p}", [128, 256], F32R)
        sbuf(f"qi_{p}", [128, 256], F32R)
        sbuf(f"vtab_{p}", [64, 1024], F32R)
        sbuf(f"zs_{p}", [128, 8, 64], F32R)
        sbuf(f"s_{p}", [128, 512], F32)
        sbuf(f"c_{p}", [128, 512], F32)
        sbuf(f"wp_{p}", [128, 512], F32R)
        sbuf(f"wtab_{p}", [64, 1024], F32R)
        sbuf(f"outt_{p}", [64, 8, 64, 2], F32)
    # ps12 doubles as psO: M1 writes/t reads early in the tick, M4 writes/outt
    # reads late (M4 waits t-done of the same tick).
    ps12 = psum("ps12", [128, 8, 64], F32)
    psT = psum("psT", [64, 1024], F32R)
    psZ = psum("psZ", [128, 8, 64], F32)
    psW = psum("psW", [128, 512], F32)
    psWsw = psum("psWsw", [128, 512], F32)
    psWt = psum("psWt", [64, 1024], F32R)
    psP1 = ps12[:, 0:4, :]
    psP2 = ps12[:, 4:8, :]

    cn = Counts()
    prog = {k: [] for k in ("sp", "pe", "dve", "act", "pool")}
    DMA_KEYS = ("in0", "in1", "out0", "out1", "cdma")

    def emit(eng, fn, sem_key):
        prog[eng].append(("op", fn, sem_key))
        return cn.inc(sem_key, 16 if sem_key in DMA_KEYS else 1)

    def wait(eng, sem_key, val):
        if val > 0:
            prog[eng].append(("wait", sem_key, val))

    def coef(i):
        return ccoef[:, 512 * i : 512 * i + 512]

    emit("sp", lambda e: e.dma_start(out=cmats_f[:, :], in_=mats_ext[:, :]), "cdma")
    d_all = emit("sp", lambda e: e.dma_start(out=ccoef[:, :], in_=coef_ext[:, :]), "cdma")
    wait("act", "cdma", d_all)
    conv_done = 0
    for i in range(10):
        def cp(e, i=i):
            return e.copy(matr[i][:, :], cmats_f[:, 128 * i : 128 * i + 128])
        conv_done = emit("act", cp, "act")
    (cM1_frT, cM1_nfiT, cM1_fiT, cM3, cM3sw, cM2a, cM2b, I128, cM4a, cM4b) = matr
    cA2, cTr, cT1i, cT1i_neg, cT2c, cD2 = [coef(i) for i in range(6)]
    cTrv = cTr.rearrange("p (r k) -> p r k", k=64)
    cT1iv = cT1i.rearrange("p (r k) -> p r k", k=64)
    cD2v = cD2.rearrange("p (r k) -> p r k", k=64)

    ev = {}
    wait("dve", "cdma", d_all)
    wait("pool", "cdma", d_all)
    wait("pe", "act", conv_done)

    def T(p, nm):
        return sb[f"{nm}_{p}"]

    # ---- per-stage emitters (group g) ----
    def st_in(g):
        p = g % 2
        if g >= 2:
            wait("sp", "pool", ev[g - 2, "Adone"])
        gd = g % ngroups_data
        last = 0
        for pr in range(4):
            def din(e, gd=gd, pr=pr, u2=T(p, "u2")):
                xpr = x_ext[
                    8 * gd + 2 * pr : 8 * gd + 2 * pr + 2, :, :, :
                ].rearrange("h n1 n2 ri -> (h n1) n2 ri")
                return e.dma_start(out=u2[:, pr, :, :], in_=xpr)
            last = emit("sp", din, f"in{p}")
        ev[g, "in"] = last

    def st_A(g):
        p = g % 2
        if g >= 2:
            wait("pool", "pe", ev[g - 2, "M1"])
        wait("pool", f"in{p}", ev[g, "in"])
        last = 0
        for pr in range(4):
            def a_re(e, pr=pr, u2=T(p, "u2"), o=T(p, "u2re")):
                return e.tensor_tensor(
                    o[:, pr, :], u2[:, pr, :, 0], cA2[:, 64 * pr : 64 * pr + 64], MUL)
            def a_im(e, pr=pr, u2=T(p, "u2"), o=T(p, "u2im")):
                return e.tensor_tensor(
                    o[:, pr, :], u2[:, pr, :, 1], cA2[:, 256 + 64 * pr : 256 + 64 * pr + 64], MUL)
            emit("pool", a_re, "pool")
            last = emit("pool", a_im, "pool")
        ev[g, "Adone"] = last

    def st_M1(g):
        p = g % 2
        if g >= 1:
            wait("pe", "dve", ev[g - 1, "T"])      # ps12 WAR: t-ops of g-1
        if g >= 8:
            wait("pe", "act", ev[g - 8, "outt"])   # ps12 WAR: outt of g-8 (prev tick)
        wait("pe", "pool", ev[g, "Adone"])
        re_c = T(p, "u2re")[:, :, :]
        im_c = T(p, "u2im")[:, :, :]
        emit("pe", lambda e, r=re_c: e.matmul(psP1, cM1_frT[:, :], r, start=True, stop=False), "pe")
        emit("pe", lambda e, i=im_c: e.matmul(psP1, cM1_nfiT[:, :], i, start=False, stop=False), "pe")
        emit("pe", lambda e, r=re_c: e.matmul(psP2, cM1_fiT[:, :], r, start=False, stop=False), "pe")
        ev[g, "M1"] = emit("pe", lambda e, i=im_c: e.matmul(psP2, cM1_frT[:, :], i, start=False, stop=True), "pe")

    def st_T(g):
        p = g % 2
        if g >= 2:
            wait("dve", "pool", ev[g - 2, "Q"])
        wait("dve", "pe", ev[g, "M1"])
        emit("dve", lambda e, o=T(p, "t14"): e.tensor_tensor(
            o[:, :].rearrange("q (r k) -> q r k", k=64), ps12[:, :, :], cTrv, MUL), "dve")
        ev[g, "T"] = emit("dve", lambda e, o=T(p, "t23"): e.tensor_tensor(
            o[:, :].rearrange("q (r k) -> q r k", k=64), ps12[:, :, :], cT1iv, MUL), "dve")

    def st_Q(g):
        p = g % 2
        if g >= 2:
            wait("pool", "pe", ev[g - 2, "Tp1"])
        wait("pool", "dve", ev[g, "T"])
        emit("pool", lambda e, qr=T(p, "qr"), t14=T(p, "t14"), t23=T(p, "t23"):
             e.tensor_tensor(qr[:, :], t14[:, 0:256], t23[:, 256:512], ADD), "pool")
        ev[g, "Q"] = emit("pool", lambda e, qi=T(p, "qi"), t14=T(p, "t14"), t23=T(p, "t23"):
             e.tensor_tensor(qi[:, :], t23[:, 0:256], t14[:, 256:512], ADD), "pool")

    def st_Tp1(g):
        p = g % 2
        if g >= 1:
            wait("pe", "act", ev[g - 1, "vt"])     # psT WAR
        wait("pe", "pool", ev[g, "Q"])
        for pr in range(4):
            def tpa(e, pr=pr, qr=T(p, "qr")):
                return e.matmul(psT[:, 128 * pr : 128 * pr + 128], qr[:, 64 * pr : 64 * pr + 64],
                                I128[:, :], is_transpose=True, start=(pr == 0), stop=(pr == 3))
            ev[g, "Tp1a"] = emit("pe", tpa, "pe")
        for pr in range(4):
            def tpb(e, pr=pr, qi=T(p, "qi")):
                return e.matmul(psT[:, 512 + 128 * pr : 512 + 128 * pr + 128], qi[:, 64 * pr : 64 * pr + 64],
                                I128[:, :], is_transpose=True, start=(pr == 0), stop=(pr == 3))
            ev[g, "Tp1"] = emit("pe", tpb, "pe")

    def st_vt(g):
        p = g % 2
        if g >= 2:
            wait("act", "pe", ev[g - 2, "M2"])     # vtab WAR
        wait("act", "pe", ev[g, "Tp1a"])
        ev[g, "vta"] = emit("act", lambda e, o=T(p, "vtab"): e.copy(o[:, 0:512], psT[:, 0:512]), "act")
        wait("act", "pe", ev[g, "Tp1"])
        ev[g, "vt"] = emit("act", lambda e, o=T(p, "vtab"): e.copy(o[:, 512:1024], psT[:, 512:1024]), "act")

    def st_M2(g):
        p = g % 2
        if g >= 1:
            wait("pe", "dve", ev[g - 1, "zs"])     # psZ WAR
        wait("pe", "act", ev[g, "vta"])
        emit("pe", lambda e, v=T(p, "vtab"): e.matmul(
            psZ[:, :, :], cM2a[0:64, :], v[:, 0:512], start=True, stop=False), "pe")
        wait("pe", "act", ev[g, "vt"])
        ev[g, "M2"] = emit("pe", lambda e, v=T(p, "vtab"): e.matmul(
            psZ[:, :, :], cM2b[0:64, :], v[:, 512:1024], start=False, stop=True), "pe")

    def st_zs(g):
        p = g % 2
        if g >= 2:
            wait("dve", "pe", ev[g - 2, "M3sw"])   # zs tile WAR (M3/M3sw read)
        wait("dve", "pe", ev[g, "M2"])
        ev[g, "zs"] = emit("dve", lambda e, o=T(p, "zs"): e.tensor_tensor(
            o[:, :, :], psZ[:, :, :], cD2v, MUL), "dve")

    def st_M3(g):
        p = g % 2
        if g >= 1:
            wait("pe", "dve", ev[g - 1, "sc"])     # psW/psWsw WAR (s_/c_ read)
        wait("pe", "dve", ev[g, "zs"])
        emit("pe", lambda e, z=T(p, "zs"): e.matmul(
            psW[:, :], cM3[:, :], z[:, :, :], start=True, stop=True), "pe")
        ev[g, "M3sw"] = emit("pe", lambda e, z=T(p, "zs"): e.matmul(
            psWsw[:, :], cM3sw[:, :], z[:, :, :], start=True, stop=True), "pe")

    def st_sc(g):
        p = g % 2
        if g >= 2:
            wait("dve", "pool", ev[g - 2, "wp"])   # s_/c_ tile WAR
        wait("dve", "pe", ev[g, "M3sw"])
        emit("dve", lambda e, o=sb[f"s_{p}"]: e.tensor_tensor(o[:, :], psW[:, :], cTr, MUL), "dve")
        ev[g, "sc"] = emit("dve", lambda e, o=sb[f"c_{p}"]: e.tensor_tensor(o[:, :], psWsw[:, :], cT2c, MUL), "dve")

    def st_wp(g):
        p = g % 2
        if g >= 2:
            wait("pool", "pe", ev[g - 2, "Tp2"])   # wp tile WAR
        wait("pool", "dve", ev[g, "sc"])
        ev[g, "wp"] = emit("pool", lambda e, o=T(p, "wp"), s_=sb[f"s_{p}"], c_=sb[f"c_{p}"]:
             e.tensor_tensor(o[:, :], s_[:, :], c_[:, :], ADD), "pool")

    def st_Tp2(g):
        p = g % 2
        if g >= 1:
            wait("pe", "act", ev[g - 1, "wt"])     # psWt WAR
        wait("pe", "pool", ev[g, "wp"])
        for r in range(8):
            def tpc(e, r=r, wp=T(p, "wp")):
                return e.matmul(psWt[:, 64 * r : 64 * r + 64], wp[0:64, 64 * r : 64 * r + 64],
                                I128[0:64, 0:64], is_transpose=True, start=(r == 0), stop=(r == 7))
            ev[g, "Tp2a"] = emit("pe", tpc, "pe")
        for r in range(8):
            def tpd(e, r=r, wp=T(p, "wp")):
                return e.matmul(psWt[:, 512 + 64 * r : 512 + 64 * r + 64], wp[64:128, 64 * r : 64 * r + 64],
                                I128[64:128, 64:128], is_transpose=True, start=(r == 0), stop=(r == 7))
            ev[g, "Tp2"] = emit("pe", tpd, "pe")

    def st_wt(g):
        p = g % 2
        if g >= 2:
            wait("act", "pe", ev[g - 2, "M4"])     # wtab WAR
        wait("act", "pe", ev[g, "Tp2"])
        ev[g, "wt"] = emit("act", lambda e, o=T(p, "wtab"): e.copy(o[:, :], psWt[:, :]), "act")

    def st_M4(g):
        p = g % 2
        # ps12 aliased: this tick's t-ops (g+7) and last tick's outt (g-1)
        # must have read ps12 before M4 overwrites it
        if (g + 7, "T") in ev:
            wait("pe", "dve", ev[g + 7, "T"])
        if g >= 1:
            wait("pe", "act", ev[g - 1, "outt"])
        wait("pe", "act", ev[g, "wt"])
        emit("pe", lambda e, w=T(p, "wtab"): e.matmul(
            ps12[:, :, :], cM4a[0:64, :], w[:, 0:512], start=True, stop=False), "pe")
        ev[g, "M4"] = emit("pe", lambda e, w=T(p, "wtab"): e.matmul(
            ps12[:, :, :], cM4b[0:64, :], w[:, 512:1024], start=False, stop=True), "pe")

    def st_outt(g):
        p = g % 2
        if g >= 2:
            wait("act", f"out{p}", ev[g - 2, "out"])  # outt tile WAR
        wait("act", "pe", ev[g, "M4"])
        emit("act", lambda e, o=T(p, "outt"): e.copy(o[:, :, :, 0], ps12[0:64, :, :]), "act")
        ev[g, "outt"] = emit("act", lambda e, o=T(p, "outt"): e.copy(o[:, :, :, 1], ps12[64:128, :, :]), "act")

    def st_out(g):
        p = g % 2
        gd = g % ngroups_data
        wait("sp", "act", ev[g, "outt"])
        def dout(e, gd=gd, outt=T(p, "outt")):
            og = out_ext[8 * gd : 8 * gd + 8, :, :].rearrange("r m2 f -> m2 r f")
            return e.dma_start(out=og, in_=outt[:, :, :, :].rearrange("p r m1 ri -> p r (m1 ri)"))
        ev[g, "out"] = emit("sp", dout, f"out{p}")

    # ---- skewed emission: stage s of group g runs in tick g + d[s] ----
    stages = [
        (0, st_in), (1, st_A), (2, st_M1), (2, st_T), (3, st_Q),
        (4, st_Tp1), (4, st_vt), (8, st_Tp2), (8, st_wt),
        (5, st_M2), (5, st_zs), (6, st_M3), (6, st_sc), (7, st_wp),
        (9, st_M4), (9, st_outt), (10, st_out),
    ]
    # engine-friendly within-tick order: PE producers early relative to their
    # same-tick consumers is handled by per-engine stream order; stage list
    # order above interleaves engines correctly (emission order only matters
    # per engine).
    for k in range(ngroups + 11):
        for d, fn in stages:
            g = k - d
            if 0 <= g < ngroups:
                fn(g)

    for key in ("in0", "in1", "out0", "out1"):
        wait("sp", key, cn.c[key])

    with (
        nc.Block() as block,
        nc.semaphore("pe_sem") as pe_sem,
        nc.semaphore("dve_sem") as dve_sem,
        nc.semaphore("act_sem") as act_sem,
        nc.semaphore("pool_sem") as pool_sem,
        nc.semaphore("in0_sem") as in0_sem,
        nc.semaphore("in1_sem") as in1_sem,
        nc.semaphore("out0_sem") as out0_sem,
        nc.semaphore("out1_sem") as out1_sem,
        nc.semaphore("cdma_sem") as cdma_sem,
    ):
        sems = {"pe": pe_sem, "dve": dve_sem, "act": act_sem, "pool": pool_sem,
                "in0": in0_sem, "in1": in1_sem, "out0": out0_sem, "out1": out1_sem,
                "cdma": cdma_sem}

        def runner(eng_name):
            def body(e):
                for item in prog[eng_name]:
                    if item[0] == "drain":
                        e.drain()
                    elif item[0] == "wait":
                        e.wait_ge(sems[item[1]], item[2])
                    else:
                        _, fn, sem_key = item
                        inst = fn(e)
                        inst.then_inc(sems[sem_key], 16 if sem_key in DMA_KEYS else 1)
            return body

        block.sync(runner("sp"))
        block.tensor(runner("pe"))
        block.vector(runner("dve"))
        block.scalar(runner("act"))
        block.gpsimd(runner("pool"))

    ctx.close()
    return nc


_NC_CACHE = {}


def kernel(x, A, D):
    """x: (8192, 4096, 2) f32; A, D: (1, 4096, 2) f32 -> (8192, 4096, 2) f32."""
    x = np.asarray(x, dtype=np.float32)
    A = np.asarray(A, dtype=np.float32)
    D = np.asarray(D, dtype=np.float32)
    B = x.shape[0]
    R = B // 8
    if R not in _NC_CACHE:
        _NC_CACHE[R] = build_nc_pipe(R)
    nc = _NC_CACHE[R]
    mats = make_mats()
    coefs = make_coefs(A, D)
    xs = x.reshape(8, R, 64, 64, 2)
    in_maps = [
        {"x": np.ascontiguousarray(xs[i]), "cmats_d": mats, "ccoef_d": coefs}
        for i in range(8)
    ]
    res = run_bass_kernel_spmd(nc, in_maps, core_ids=list(range(8)))
    out = np.concatenate(
        [res.results[i]["out"].reshape(R, 4096, 2) for i in range(8)], axis=0
    )
    return out



# revision 2
# speedup vs baseline: 1.0775x; 1.0775x over previous
"""v4 Trainium2 Bass kernel: out = IFFT(D .* FFT(A .* x)), 64x64 4-step FFT,
fp16 PE path (f32r gets power-throttled to ~50%; fp16 runs at full 2.4GHz).

Per 8-row group (see dev/v4_math.py for validated math):
  M1 (PE, 4mm blk2-h, fp16)  -> psP  [128=(h,d), (ri|., pr, b) 512] f32
  t  (DVE, 2 ops)            -> t2   [128, 1024] fp16 = [psP|psP] .* cT
  Tp1(PE, 16 transp, accum)  -> psTa/psTb [64, 512] (q-adds folded into accum)
  vt (ACT, 2 copies)         -> vtab [128=(ri_b,b), (pr,h,d) 512] fp16
  M2 (PE, 1mm dense)         -> psF  [128=(ri_c,c), (pr,h,d) 512]
  zs (DVE, 1 op, D-scale)    -> zsb  [128, 512] fp16
  M3 (PE, 2mm dense+sw)      -> psW2 [128, 1024] = [U | Usw]
  s  (DVE, 1 op)             -> s2   [128, 1024] fp16
  wp (pool, 1 add)           -> wpb  [128=(ri_g,g), (pr,h,d) 512] fp16
  Tp2(PE, 8 transp)          -> psWt [128=(h,d), (pr,ri,g) 512]
  wt (ACT, 1 copy)           -> wtab fp16
  M4 (PE, 4mm blk2-h)        -> psO  [128=(h,e), (ri_slot, pr, g) 512]
  outt(ACT, 2 copies)        -> outt [128=(h,e), (pr, g, ri)] f32 -> DMA out
"""
import sys

for p in ("/opt/trn_rl_repo",):
    if p not in sys.path:
        sys.path.insert(0, p)

import numpy as np
from concourse import bass, mybir

F32 = mybir.dt.float32
F16 = mybir.dt.float16
MUL = mybir.AluOpType.mult
ADD = mybir.AluOpType.add
GROUP = 8
N = 4096
R64 = 64


# ---------------- constants (host) ----------------

def _f64():
    a = np.arange(R64)
    return np.exp(-2j * np.pi * np.outer(a, a) / R64)


def _blk2(M):
    out = np.zeros((128, 128))
    out[:64, :64] = M
    out[64:, 64:] = M
    return out


def make_mats2():
    F = _f64()
    Fr, Fi = F.real, F.imag
    G = np.conj(F) / R64
    Gr, Gi = G.real, G.imag
    mats = np.concatenate([
        _blk2(Fr), _blk2(-Fi), _blk2(Fi),
        np.block([[Fr, Fi], [-Fi, Fr]]),
        np.block([[Gr, Gi], [-Gi, Gr]]),
        np.block([[Gi, Gr], [Gr, -Gi]]),
        np.eye(128),
        _blk2(Gr), _blk2(-Gi), _blk2(Gi),
    ], axis=1)
    return mats.astype(np.float32)


def make_coefs2(A, D):
    A2 = np.asarray(A, np.float32).reshape(64, 64, 2)
    D2 = np.asarray(D, np.float32).reshape(64, 64, 2)
    t2r = lambda M: np.tile(np.vstack([M, M]), (1, 4))
    t8 = lambda M: np.tile(M, (1, 8))

    cA = np.concatenate([t2r(A2[:, :, 0]), t2r(A2[:, :, 1])], axis=1)

    d_ = np.arange(64)[:, None]
    b_ = np.arange(64)[None, :]
    th = 2 * np.pi * (d_ * b_) / N
    twr, twi = np.cos(th), -np.sin(th)
    cT14 = np.tile(np.vstack([twr, twr]), (1, 8))
    cT23 = np.concatenate(
        [np.tile(np.vstack([twi, twi]), (1, 4)),
         np.tile(np.vstack([-twi, -twi]), (1, 4))], axis=1)
    cT = np.concatenate([cT14, cT23], axis=1)

    cD = np.vstack([t8(D2[:, :, 0]), t8(D2[:, :, 1])])

    ph = 2 * np.pi * (d_ * b_) / N  # rows=g, cols=d
    itwr, itwi = np.cos(ph), np.sin(ph)
    cS = np.concatenate(
        [np.vstack([t8(itwr), t8(itwr)]), np.vstack([t8(-itwi), t8(itwi)])],
        axis=1)

    return np.concatenate([cA, cT, cD, cS], axis=1).astype(np.float32)


# ---------------- kernel build ----------------

class Counts:
    def __init__(self):
        self.c = {"pe": 0, "dve": 0, "act": 0, "pool": 0,
                  "in0": 0, "in1": 0, "out0": 0, "out1": 0, "cdma": 0}

    def inc(self, k, n=1):
        self.c[k] += n
        return self.c[k]


DMA_KEYS = ("in0", "in1", "out0", "out1", "cdma")


def build_nc_v4(R, nrep=1, cut=None):
    """cut: emit only the first `cut` pipeline stages (debug bisection)."""
    assert R % GROUP == 0
    ngroups_data = R // GROUP
    ngroups = ngroups_data * nrep
    nc = bass.Bass("TRN2", target_bir_lowering=False)

    x_ext = nc.declare_dram_parameter("x", [R, 64, 64, 2], F32, isOutput=False)
    mats_ext = nc.declare_dram_parameter("cmats_d", [128, 1280], F32, isOutput=False)
    coef_ext = nc.declare_dram_parameter("ccoef_d", [128, 3072], F32, isOutput=False)
    out_ext = nc.declare_dram_parameter("out", [R, 64, 64, 2], F32, isOutput=True)

    from contextlib import ExitStack
    ctx = ExitStack()
    sb = {}

    def sbuf(name, shape, dt):
        sb[name] = ctx.enter_context(nc.sbuf_tensor(name, shape, dt))
        return sb[name]

    def psum(name, shape, dt):
        sb[name] = ctx.enter_context(nc.psum_tensor(name, shape, dt))
        return sb[name]

    cmats_f = sbuf("cmats_f", [128, 1280], F32)
    mats16 = sbuf("mats16", [128, 1280], F16)
    ccoef = sbuf("ccoef", [128, 3072], F32)
    m0 = mats16[:, 0:128]
    m1 = mats16[:, 128:256]
    m2 = mats16[:, 256:384]
    S2 = mats16[:, 384:512]
    S3 = mats16[:, 512:640]
    S3sw = mats16[:, 640:768]
    I128 = mats16[:, 768:896]
    m7 = mats16[:, 896:1024]
    m8 = mats16[:, 1024:1152]
    m9 = mats16[:, 1152:1280]
    cA = ccoef[:, 0:512]
    cT = ccoef[:, 512:1536]
    cD = ccoef[:, 1536:2048]
    cS = ccoef[:, 2048:3072]

    for p in range(2):
        sbuf(f"u2_{p}", [128, 4, 64, 2], F32)
        sbuf(f"t2_{p}", [128, 1024], F16)
        sbuf(f"qrt_{p}", [128, 512], F16)
        sbuf(f"vtab_{p}", [128, 512], F16)
        sbuf(f"zsb_{p}", [128, 512], F16)
        sbuf(f"s2_{p}", [128, 1024], F16)
        sbuf(f"wpb_{p}", [128, 512], F16)
        sbuf(f"wtab_{p}", [128, 512], F16)
        sbuf(f"outt_{p}", [128, 4, 64, 2], F32)
    for p in range(3):
        sbuf(f"u2re_{p}", [128, 4, 64], F16)
        sbuf(f"u2im_{p}", [128, 4, 64], F16)

    psP0 = psum("psP0", [128, 512], F32)
    psP1 = psum("psP1", [128, 512], F32)
    psPs = [psP0, psP1]
    psT = psum("psT", [128, 512], F16)
    psF = psum("psF", [128, 512], F32)
    psW2 = psum("psW2", [128, 1024], F32)
    psWt = psum("psWt", [128, 512], F16)
    psO = psum("psO", [128, 512], F32)

    cn = Counts()
    prog = {k: [] for k in ("sp", "pe", "dve", "act", "pool")}

    def emit(eng, fn, sem_key):
        prog[eng].append(("op", fn, sem_key))
        return cn.inc(sem_key, 16 if sem_key in DMA_KEYS else 1)

    def wait(eng, sem_key, val):
        if val > 0:
            prog[eng].append(("wait", sem_key, val))

    # constants: DMA f32, convert stationaries to fp16 on ACT once
    emit("sp", lambda e: e.dma_start(out=cmats_f[:, :], in_=mats_ext[:, :]), "cdma")
    d_all = emit("sp", lambda e: e.dma_start(out=ccoef[:, :], in_=coef_ext[:, :]), "cdma")
    wait("act", "cdma", d_all)
    conv_done = emit("act", lambda e: e.copy(mats16[:, :], cmats_f[:, :]), "act")
    wait("pe", "act", conv_done)
    wait("dve", "cdma", d_all)
    wait("pool", "cdma", d_all)

    ev = {}

    def T(p, nm):
        return sb[f"{nm}_{p}"]

    def st_in(g):
        p = g % 2
        gd = g % ngroups_data
        if g >= 2:
            wait("sp", "pool", ev[g - 2, "A"])
        last = 0
        for q in range(2):
            def din(e, gd=gd, q=q, u2=T(p, "u2")):
                xq = x_ext[
                    8 * gd + 4 * q: 8 * gd + 4 * q + 4, :, :, :
                ].rearrange("(prp h) a b ri -> (h a) prp (b ri)", h=2)
                return e.dma_start(
                    out=u2[:, 2 * q: 2 * q + 2, :, :].rearrange(
                        "p pr b ri -> p pr (b ri)"),
                    in_=xq)
            last = emit("sp", din, f"in{p}")
        ev[g, "in"] = last

    def st_A(g):
        p = g % 2
        p3 = g % 3
        wait("pool", f"in{p}", ev[g, "in"])
        if g >= 3:
            wait("pool", "pe", ev[g - 3, "M1"])
        cAre = cA[:, 0:256].rearrange("p (pr b) -> p pr b", b=64)
        cAim = cA[:, 256:512].rearrange("p (pr b) -> p pr b", b=64)
        emit("pool", lambda e, u2=T(p, "u2"), o=T(p3, "u2re"), c=cAre:
             e.tensor_tensor(o[:, :, :], u2[:, :, :, 0], c, MUL), "pool")
        ev[g, "A"] = emit("pool", lambda e, u2=T(p, "u2"), o=T(p3, "u2im"), c=cAim:
                          e.tensor_tensor(o[:, :, :], u2[:, :, :, 1], c, MUL), "pool")

    def st_M1(g):
        p3 = g % 3
        psP = psPs[g % 2]
        wait("pe", "pool", ev[g, "A"])
        if g >= 2:
            wait("pe", "dve", ev[g - 2, "t"])
        # ordered to minimize stationary reloads: m2, m0, m0, m1
        re = T(p3, "u2re").rearrange("p pr b -> p (pr b)")
        im = T(p3, "u2im").rearrange("p pr b -> p (pr b)")
        emit("pe", lambda e, re=re, o=psP: e.matmul(o[:, 256:512], m2, re, start=True, stop=False), "pe")
        emit("pe", lambda e, im=im, o=psP: e.matmul(o[:, 256:512], m0, im, start=False, stop=True), "pe")
        emit("pe", lambda e, re=re, o=psP: e.matmul(o[:, 0:256], m0, re, start=True, stop=False), "pe")
        ev[g, "M1"] = emit("pe", lambda e, im=im, o=psP: e.matmul(o[:, 0:256], m1, im, start=False, stop=True), "pe")

    def st_t(g):
        p = g % 2
        psP = psPs[g % 2]
        wait("dve", "pe", ev[g, "M1"])
        if g >= 2:
            wait("dve", "pool", ev[g - 2, "q"])
        ev[g, "t"] = emit("dve", lambda e, o=T(p, "t2"), i=psP: e.tensor_tensor(
            o[:, :].rearrange("p (r c) -> p r c", r=2),
            i[:, :].unsqueeze(1).broadcast_to((128, 2, 512)),
            cT.rearrange("p (r c) -> p r c", r=2), MUL), "dve")

    def st_q(g):
        # qr = t14_re + t23_im ; qi = t23_re + t14_im (signs baked in cT)
        p = g % 2
        wait("pool", "dve", ev[g, "t"])
        if g >= 2:
            wait("pool", "pe", ev[g - 2, "Tp1"])
        emit("pool", lambda e, t2=T(p, "t2"), o=T(p, "qrt"): e.tensor_tensor(
            o[:, 0:256], t2[:, 0:256], t2[:, 768:1024], ADD), "pool")
        ev[g, "q"] = emit("pool", lambda e, t2=T(p, "t2"), o=T(p, "qrt"): e.tensor_tensor(
            o[:, 256:512], t2[:, 512:768], t2[:, 256:512], ADD), "pool")

    def st_Tp1(g):
        p = g % 2
        wait("pe", "pool", ev[g, "q"])
        if g >= 1:
            wait("pe", "act", ev[g - 1, "vt"])
        q = T(p, "qrt")
        for pr in range(4):
            emit("pe", lambda e, q=q, pr=pr: e.matmul(
                psT[0:64, 128 * pr:128 * pr + 128], q[:, 64 * pr:64 * pr + 64],
                I128, is_transpose=True, start=True, stop=True), "pe")
            ev[g, "Tp1"] = emit("pe", lambda e, q=q, pr=pr: e.matmul(
                psT[64:128, 128 * pr:128 * pr + 128], q[:, 256 + 64 * pr:256 + 64 * pr + 64],
                I128, is_transpose=True, start=True, stop=True), "pe")

    def st_vt(g):
        p = g % 2
        wait("act", "pe", ev[g, "Tp1"])
        if g >= 2:
            wait("act", "pe", ev[g - 2, "M2"])
        ev[g, "vt"] = emit("act", lambda e, o=T(p, "vtab"): e.copy(o[:, :], psT[:, :]), "act")

    def st_M2(g):
        p = g % 2
        wait("pe", "act", ev[g, "vt"])
        if g >= 1:
            wait("pe", "dve", ev[g - 1, "zs"])
        ev[g, "M2"] = emit("pe", lambda e, v=T(p, "vtab"): e.matmul(
            psF[:, :], S2, v[:, :], start=True, stop=True), "pe")

    def st_zs(g):
        p = g % 2
        wait("dve", "pe", ev[g, "M2"])
        if g >= 2:
            wait("dve", "pe", ev[g - 2, "M3"])
        ev[g, "zs"] = emit("dve", lambda e, o=T(p, "zsb"): e.tensor_tensor(
            o[:, :], psF[:, :], cD, MUL), "dve")

    def st_M3(g):
        p = g % 2
        wait("pe", "dve", ev[g, "zs"])
        if g >= 1:
            wait("pe", "dve", ev[g - 1, "s"])
        emit("pe", lambda e, z=T(p, "zsb"): e.matmul(
            psW2[:, 0:512], S3, z[:, :], start=True, stop=True), "pe")
        ev[g, "M3"] = emit("pe", lambda e, z=T(p, "zsb"): e.matmul(
            psW2[:, 512:1024], S3sw, z[:, :], start=True, stop=True), "pe")

    def st_s(g):
        p = g % 2
        wait("dve", "pe", ev[g, "M3"])
        # s2 WAR vs wp(g-2): same engine (DVE), in program order
        ev[g, "s"] = emit("dve", lambda e, o=T(p, "s2"): e.tensor_tensor(
            o[:, :], psW2[:, :], cS, MUL), "dve")

    def st_wp(g):
        # on DVE: all-fp16 SBUF op -> 2x mode; self-wait for s(g) writeback
        p = g % 2
        wait("dve", "dve", ev[g, "s"])
        if g >= 2:
            wait("dve", "pe", ev[g - 2, "Tp2"])
        ev[g, "wp"] = emit("dve", lambda e, o=T(p, "wpb"), s=T(p, "s2"):
                           e.tensor_tensor(o[:, :], s[:, 0:512], s[:, 512:1024], ADD), "dve")

    def st_Tp2(g):
        p = g % 2
        wait("pe", "dve", ev[g, "wp"])
        if g >= 1:
            wait("pe", "act", ev[g - 1, "wt"])
        # full-block transposes: psWt[:,128pr:+128] = wp-block^T
        # psWt cols become (pr, ri, g)
        wpb = T(p, "wpb")
        for pr in range(4):
            ev[g, "Tp2"] = emit("pe", lambda e, w=wpb, pr=pr: e.matmul(
                psWt[:, 128 * pr:128 * pr + 128], w[:, 128 * pr:128 * pr + 128],
                I128, is_transpose=True, start=True, stop=True), "pe")

    def st_wt(g):
        p = g % 2
        wait("act", "pe", ev[g, "Tp2"])
        if g >= 2:
            wait("act", "pe", ev[g - 2, "M4"])
        ev[g, "wt"] = emit("act", lambda e, o=T(p, "wtab"): e.copy(
            o[:, :], psWt[:, :]), "act")

    def st_M4(g):
        p = g % 2
        wait("pe", "act", ev[g, "wt"])
        if g >= 1:
            wait("pe", "act", ev[g - 1, "outt"])
        # wtab cols (pr, ri, g): 4 strided-moving matmuls (one reload each)
        w = T(p, "wtab").rearrange("p (pr ri g) -> p pr ri g", ri=2, g=64)
        vr = w[:, :, 0, :]
        vi = w[:, :, 1, :]
        emit("pe", lambda e, v=vr: e.matmul(psO[:, 0:256], m7, v, start=True, stop=False), "pe")
        emit("pe", lambda e, v=vi: e.matmul(psO[:, 0:256], m8, v, start=False, stop=True), "pe")
        emit("pe", lambda e, v=vr: e.matmul(psO[:, 256:512], m9, v, start=True, stop=False), "pe")
        ev[g, "M4"] = emit("pe", lambda e, v=vi: e.matmul(psO[:, 256:512], m7, v, start=False, stop=True), "pe")

    def st_outt(g):
        p = g % 2
        wait("act", "pe", ev[g, "M4"])
        if g >= 2:
            wait("act", f"out{p}", ev[g - 2, "out"])
        pre = psO[:, 0:256].rearrange("p (pr g) -> p pr g", g=64)
        pim = psO[:, 256:512].rearrange("p (pr g) -> p pr g", g=64)
        emit("act", lambda e, o=T(p, "outt"), i=pre: e.copy(o[:, :, :, 0], i), "act")
        ev[g, "outt"] = emit("act", lambda e, o=T(p, "outt"), i=pim: e.copy(o[:, :, :, 1], i), "act")

    def st_out(g):
        p = g % 2
        gd = g % ngroups_data
        wait("sp", "act", ev[g, "outt"])
        def dout(e, gd=gd, outt=T(p, "outt")):
            og = out_ext[8 * gd: 8 * gd + 8, :, :, :].rearrange(
                "(pr h) eo g ri -> (h eo) pr (g ri)", h=2)
            return e.dma_start(out=og, in_=outt[:, :, :, :].rearrange(
                "p pr g ri -> p pr (g ri)"))
        ev[g, "out"] = emit("sp", dout, f"out{p}")

    chain = [st_in, st_A, st_M1, st_t, st_q, st_Tp1, st_vt, st_M2, st_zs,
             st_M3, st_s, st_wp, st_Tp2, st_wt, st_M4, st_outt, st_out]
    enabled = set(chain if cut is None else chain[:cut])
    stages = [
        (0, st_in), (1, st_A),
        # PE order in tick: M2, M3, M1, Tp1, Tp2, M4 (DVE producers first)
        (6, st_M2), (7, st_M3), (2, st_M1), (5, st_Tp1), (9, st_Tp2), (10, st_M4),
        # DVE: t, zs, s
        (3, st_t), (6, st_zs), (7, st_s),
        # ACT: vt, wt, outt
        (5, st_vt), (9, st_wt), (10, st_outt),
        # pool: A (above), q, wp
        (4, st_q), (8, st_wp),
        (11, st_out),
    ]
    for k in range(ngroups + 12):
        for d, fn in stages:
            g = k - d
            if 0 <= g < ngroups and fn in enabled:
                fn(g)

    for key in ("in0", "in1", "out0", "out1"):
        wait("sp", key, cn.c[key])

    with (
        nc.Block() as block,
        nc.semaphore("pe_sem") as pe_sem,
        nc.semaphore("dve_sem") as dve_sem,
        nc.semaphore("act_sem") as act_sem,
        nc.semaphore("pool_sem") as pool_sem,
        nc.semaphore("in0_sem") as in0_sem,
        nc.semaphore("in1_sem") as in1_sem,
        nc.semaphore("out0_sem") as out0_sem,
        nc.semaphore("out1_sem") as out1_sem,
        nc.semaphore("cdma_sem") as cdma_sem,
    ):
        sems = {"pe": pe_sem, "dve": dve_sem, "act": act_sem, "pool": pool_sem,
                "in0": in0_sem, "in1": in1_sem, "out0": out0_sem,
                "out1": out1_sem, "cdma": cdma_sem}

        def runner(eng_name):
            def body(e):
                for item in prog[eng_name]:
                    if item[0] == "wait":
                        e.wait_ge(sems[item[1]], item[2])
                    else:
                        _, fn, sem_key = item
                        inst = fn(e)
                        inst.then_inc(sems[sem_key], 16 if sem_key in DMA_KEYS else 1)
            return body

        block.sync(runner("sp"))
        block.tensor(runner("pe"))
        block.vector(runner("dve"))
        block.scalar(runner("act"))
        block.gpsimd(runner("pool"))

    ctx.close()
    return nc


_NC_CACHE = {}


def kernel(x, A, D):
    from concourse.bass_utils import run_bass_kernel_spmd
    x = np.asarray(x, dtype=np.float32)
    B = x.shape[0]
    R = B // 8
    if R not in _NC_CACHE:
        _NC_CACHE[R] = build_nc_v4(R)
    nc = _NC_CACHE[R]
    mats = make_mats2()
    coefs = make_coefs2(A, D)
    xs = x.reshape(8, R, 64, 64, 2)
    in_maps = [
        {"x": np.ascontiguousarray(xs[i]), "cmats_d": mats, "ccoef_d": coefs}
        for i in range(8)
    ]
    res = run_bass_kernel_spmd(nc, in_maps, core_ids=list(range(8)))
    out = np.concatenate(
        [res.results[i]["out"].reshape(R, 4096, 2) for i in range(8)], axis=0
    )
    return out


# revision 4
# speedup vs baseline: 1.1052x; 1.0258x over previous
"""Trainium2 Bass kernel: out = IFFT(D .* FFT(A .* x)) over dim 1 (4096),
batch 8192, A/D scale re/im independently. Data-parallel over 8 NeuronCores.

4-step FFT (4096 = 64x64), all-fp16 PE path (f32r matmuls get
power-throttled to ~50% util on TRN2; fp16 runs at full 2.4 GHz).
Skewed 14-deep software pipeline over 8-row groups; per group:

  in  (SP, 2 DMAs)          -> u2   [128=(h,a), (pr,b,ri)] f32
  A   (pool, 2 muls)        -> u2re/u2im fp16 (A-scale)
  M1  (PE, 4mm blk2-h)      -> psP  [128=(h,d), (re|im)(pr,b)] f32  x2 bufs
  pb  (ACT evict)           -> pb16 fp16   (enables DVE 2x for t)
  t   (DVE 1 op, 2x)        -> t2   [128,1024] fp16 = [psP|psP].*twiddle1
  q   (pool, 2 adds)        -> qrt  [128=(h,d), (qr|qi)(pr,b)] fp16
  Tp1 (PE, 8 transp)        -> psT  [128=(ri_b,b), (pr,h,d)] fp16
  vt  (ACT, 1 copy)         -> vtab fp16
  M2  (PE, 1mm dense)       -> psF  [128=(ri_c,c), (pr,h,d)] f32
  zs  (DVE, D-scale)        -> zsb  fp16
  M3  (PE, 2mm dense+sw)    -> psW2 [128,1024] = [U | Usw] f32
  s   (DVE, twiddle2)       -> s2   [128,1024] fp16
  wp  (DVE add, 2x)         -> wpb  [128=(ri_g,g), (pr,h,d)] fp16
  Tp2 (PE, 4 full transp)   -> psWt [128=(h,d), (pr,ri,g)] fp16
  wt  (ACT, 1 copy)         -> wtab fp16
  M4  (PE, 4mm blk2-h)      -> psO  [128=(h,e), (re|im)(pr,g)] f32
  outt(ACT, 2 copies)       -> outt [128=(h,e), (pr,g,ri)] f32
  out (SP, 1 DMA)           -> DRAM

Measured: ~481 us on-device for the full 8192x4096 problem (8 cores),
max-rel err ~7.5e-4 (vs 2e-2 budget).
"""
import sys

for p in ("/opt/trn_rl_repo",):
    if p not in sys.path:
        sys.path.insert(0, p)

import numpy as np
from concourse import bass, mybir

F32 = mybir.dt.float32
F16 = mybir.dt.float16
MUL = mybir.AluOpType.mult
ADD = mybir.AluOpType.add
GROUP = 8
N = 4096
R64 = 64


# ---------------- constants (host) ----------------

def _f64():
    a = np.arange(R64)
    return np.exp(-2j * np.pi * np.outer(a, a) / R64)


def _blk2(M):
    out = np.zeros((128, 128))
    out[:64, :64] = M
    out[64:, 64:] = M
    return out


def make_mats2():
    F = _f64()
    Fr, Fi = F.real, F.imag
    G = np.conj(F) / R64
    Gr, Gi = G.real, G.imag
    mats = np.concatenate([
        _blk2(Fr), _blk2(-Fi), _blk2(Fi),
        np.block([[Fr, Fi], [-Fi, Fr]]),
        np.block([[Gr, Gi], [-Gi, Gr]]),
        np.block([[Gi, Gr], [Gr, -Gi]]),
        np.eye(128),
        _blk2(Gr), _blk2(-Gi), _blk2(Gi),
    ], axis=1)
    return mats.astype(np.float32)


def make_coefs2(A, D):
    A2 = np.asarray(A, np.float32).reshape(64, 64, 2)
    D2 = np.asarray(D, np.float32).reshape(64, 64, 2)
    t2r = lambda M: np.tile(np.vstack([M, M]), (1, 4))
    t8 = lambda M: np.tile(M, (1, 8))

    cA = np.concatenate([t2r(A2[:, :, 0]), t2r(A2[:, :, 1])], axis=1)

    d_ = np.arange(64)[:, None]
    b_ = np.arange(64)[None, :]
    th = 2 * np.pi * (d_ * b_) / N
    twr, twi = np.cos(th), -np.sin(th)
    cT14 = np.tile(np.vstack([twr, twr]), (1, 8))
    cT23 = np.concatenate(
        [np.tile(np.vstack([twi, twi]), (1, 4)),
         np.tile(np.vstack([-twi, -twi]), (1, 4))], axis=1)
    cT = np.concatenate([cT14, cT23], axis=1)

    cD = np.vstack([t8(D2[:, :, 0]), t8(D2[:, :, 1])])

    ph = 2 * np.pi * (d_ * b_) / N  # rows=g, cols=d
    itwr, itwi = np.cos(ph), np.sin(ph)
    cS = np.concatenate(
        [np.vstack([t8(itwr), t8(itwr)]), np.vstack([t8(-itwi), t8(itwi)])],
        axis=1)

    return np.concatenate([cA, cT, cD, cS], axis=1).astype(np.float32)


# ---------------- kernel build ----------------

class Counts:
    def __init__(self):
        self.c = {"pe": 0, "dve": 0, "act": 0, "pool": 0,
                  "in0": 0, "in1": 0, "out0": 0, "out1": 0, "cdma": 0,
                  "xb0": 0, "xb1": 0}

    def inc(self, k, n=1):
        self.c[k] += n
        return self.c[k]


DMA_KEYS = ("in0", "in1", "out0", "out1", "cdma", "xb0", "xb1")


def build_nc_v4(R, nrep=1, cut=None):
    """cut: emit only the first `cut` pipeline stages (debug bisection)."""
    assert R % GROUP == 0
    ngroups_data = R // GROUP
    ngroups = ngroups_data * nrep
    nc = bass.Bass("TRN2", target_bir_lowering=False)

    x_ext = nc.declare_dram_parameter("x", [R, 64, 64, 2], F32, isOutput=False)
    mats_ext = nc.declare_dram_parameter("cmats_d", [128, 1280], F32, isOutput=False)
    coef_ext = nc.declare_dram_parameter("ccoef_d", [128, 3072], F32, isOutput=False)
    out_ext = nc.declare_dram_parameter("out", [R, 64, 64, 2], F32, isOutput=True)

    from contextlib import ExitStack
    ctx = ExitStack()
    sb = {}

    def sbuf(name, shape, dt):
        sb[name] = ctx.enter_context(nc.sbuf_tensor(name, shape, dt))
        return sb[name]

    def psum(name, shape, dt):
        sb[name] = ctx.enter_context(nc.psum_tensor(name, shape, dt))
        return sb[name]

    cmats_f = sbuf("cmats_f", [128, 1280], F32)
    mats16 = sbuf("mats16", [128, 1280], F16)
    ccoef = sbuf("ccoef", [128, 3072], F32)
    ccoef16 = sbuf("ccoef16", [128, 3072], F16)
    m0 = mats16[:, 0:128]
    m1 = mats16[:, 128:256]
    m2 = mats16[:, 256:384]
    S2 = mats16[:, 384:512]
    S3 = mats16[:, 512:640]
    S3sw = mats16[:, 640:768]
    I128 = mats16[:, 768:896]
    m7 = mats16[:, 896:1024]
    m8 = mats16[:, 1024:1152]
    m9 = mats16[:, 1152:1280]
    cA = ccoef[:, 0:512]
    cS = ccoef[:, 2048:3072]
    cT16 = ccoef16[:, 512:1536]
    cD16 = ccoef16[:, 1536:2048]

    for p in range(2):
        sbuf(f"u2_{p}", [128, 4, 64, 2], F32)
        sbuf(f"pb16_{p}", [128, 512], F16)
        sbuf(f"t2_{p}", [128, 1024], F16)
        sbuf(f"qrt_{p}", [128, 512], F16)
        sbuf(f"vtab_{p}", [128, 512], F16)
        sbuf(f"fb16_{p}", [128, 512], F16)
        sbuf(f"zsb_{p}", [128, 512], F16)
        sbuf(f"s2_{p}", [128, 1024], F16)
        sbuf(f"wpb_{p}", [128, 512], F16)
        sbuf(f"wtab_{p}", [128, 512], F16)
        sbuf(f"outt_{p}", [128, 4, 64, 2], F32)
    for p in range(3):
        sbuf(f"u2re_{p}", [128, 4, 64], F16)
        sbuf(f"u2im_{p}", [128, 4, 64], F16)

    psP0 = psum("psP0", [128, 512], F32)
    psP1 = psum("psP1", [128, 512], F32)
    psPs = [psP0, psP1]
    psT = psum("psT", [128, 512], F16)
    psF = psum("psF", [128, 512], F32)
    psW2 = psum("psW2", [128, 1024], F32)
    psWt = psum("psWt", [128, 512], F16)
    psO = psum("psO", [128, 512], F32)

    cn = Counts()
    prog = {k: [] for k in ("sp", "pe", "dve", "act", "pool")}

    def emit(eng, fn, sem_key):
        prog[eng].append(("op", fn, sem_key))
        return cn.inc(sem_key, 16 if sem_key in DMA_KEYS else 1)

    def wait(eng, sem_key, val):
        if val > 0:
            prog[eng].append(("wait", sem_key, val))

    # constants: DMA f32, convert stationaries+coefs to fp16 on ACT once
    emit("sp", lambda e: e.dma_start(out=cmats_f[:, :], in_=mats_ext[:, :]), "cdma")
    d_all = emit("sp", lambda e: e.dma_start(out=ccoef[:, :], in_=coef_ext[:, :]), "cdma")
    wait("act", "cdma", d_all)
    emit("act", lambda e: e.copy(mats16[:, :], cmats_f[:, :]), "act")
    conv_done = emit("act", lambda e: e.copy(ccoef16[:, :], ccoef[:, :]), "act")
    wait("pe", "act", conv_done)
    wait("dve", "act", conv_done)
    wait("pool", "cdma", d_all)

    ev = {}

    def T(p, nm):
        return sb[f"{nm}_{p}"]

    def st_in(g):
        p = g % 2
        gd = g % ngroups_data
        if g >= 2:
            wait("sp", "pool", ev[g - 2, "A"])
        last = 0
        for q in range(2):
            def din(e, gd=gd, q=q, u2=T(p, "u2")):
                xq = x_ext[
                    8 * gd + 4 * q: 8 * gd + 4 * q + 4, :, :, :
                ].rearrange("(prp h) a b ri -> (h a) prp (b ri)", h=2)
                return e.dma_start(
                    out=u2[:, 2 * q: 2 * q + 2, :, :].rearrange(
                        "p pr b ri -> p pr (b ri)"),
                    in_=xq)
            last = emit("sp", din, f"in{p}")
        ev[g, "in"] = last

    def st_A(g):
        p = g % 2
        p3 = g % 3
        wait("pool", f"in{p}", ev[g, "in"])
        if g >= 3:
            wait("pool", "pe", ev[g - 3, "M1"])
        cAre = cA[:, 0:256].rearrange("p (pr b) -> p pr b", b=64)
        cAim = cA[:, 256:512].rearrange("p (pr b) -> p pr b", b=64)
        emit("pool", lambda e, u2=T(p, "u2"), o=T(p3, "u2re"), c=cAre:
             e.tensor_tensor(o[:, :, :], u2[:, :, :, 0], c, MUL), "pool")
        ev[g, "A"] = emit("pool", lambda e, u2=T(p, "u2"), o=T(p3, "u2im"), c=cAim:
                          e.tensor_tensor(o[:, :, :], u2[:, :, :, 1], c, MUL), "pool")

    def st_M1(g):
        p3 = g % 3
        psP = psPs[g % 2]
        wait("pe", "pool", ev[g, "A"])
        if g >= 2:
            wait("pe", "act", ev[g - 2, "pb"])
        # ordered to minimize stationary reloads: m2, m0, m0, m1
        re = T(p3, "u2re").rearrange("p pr b -> p (pr b)")
        im = T(p3, "u2im").rearrange("p pr b -> p (pr b)")
        emit("pe", lambda e, re=re, o=psP: e.matmul(o[:, 256:512], m2, re, start=True, stop=False), "pe")
        emit("pe", lambda e, im=im, o=psP: e.matmul(o[:, 256:512], m0, im, start=False, stop=True), "pe")
        emit("pe", lambda e, re=re, o=psP: e.matmul(o[:, 0:256], m0, re, start=True, stop=False), "pe")
        ev[g, "M1"] = emit("pe", lambda e, im=im, o=psP: e.matmul(o[:, 0:256], m1, im, start=False, stop=True), "pe")

    def st_pb(g):
        # ACT: evict psP f32 -> fp16 SBUF so the t-op runs in DVE 2x mode
        p = g % 2
        psP = psPs[g % 2]
        wait("act", "pe", ev[g, "M1"])
        if g >= 2:
            wait("act", "dve", ev[g - 2, "t"])
        ev[g, "pb"] = emit("act", lambda e, o=T(p, "pb16"), i=psP:
                           e.copy(o[:, :], i[:, :]), "act")

    def st_t(g):
        p = g % 2
        wait("dve", "act", ev[g, "pb"])
        if g >= 2:
            wait("dve", "pool", ev[g - 2, "q"])
        ev[g, "t"] = emit("dve", lambda e, o=T(p, "t2"), i=T(p, "pb16"): e.tensor_tensor(
            o[:, :].rearrange("p (r c) -> p r c", r=2),
            i[:, :].unsqueeze(1).broadcast_to((128, 2, 512)),
            cT16.rearrange("p (r c) -> p r c", r=2), MUL), "dve")

    def st_q(g):
        # qr = t14_re + t23_im ; qi = t23_re + t14_im (signs baked in cT)
        p = g % 2
        wait("pool", "dve", ev[g, "t"])
        if g >= 2:
            wait("pool", "pe", ev[g - 2, "Tp1"])
        emit("pool", lambda e, t2=T(p, "t2"), o=T(p, "qrt"): e.tensor_tensor(
            o[:, 0:256], t2[:, 0:256], t2[:, 768:1024], ADD), "pool")
        ev[g, "q"] = emit("pool", lambda e, t2=T(p, "t2"), o=T(p, "qrt"): e.tensor_tensor(
            o[:, 256:512], t2[:, 512:768], t2[:, 256:512], ADD), "pool")

    def st_Tp1(g):
        p = g % 2
        wait("pe", "pool", ev[g, "q"])
        if g >= 1:
            wait("pe", "act", ev[g - 1, "vt"])
        q = T(p, "qrt")
        for pr in range(4):
            emit("pe", lambda e, q=q, pr=pr: e.matmul(
                psT[0:64, 128 * pr:128 * pr + 128], q[:, 64 * pr:64 * pr + 64],
                I128, is_transpose=True, start=True, stop=True), "pe")
            ev[g, "Tp1"] = emit("pe", lambda e, q=q, pr=pr: e.matmul(
                psT[64:128, 128 * pr:128 * pr + 128], q[:, 256 + 64 * pr:256 + 64 * pr + 64],
                I128, is_transpose=True, start=True, stop=True), "pe")

    def st_vt(g):
        p = g % 2
        wait("act", "pe", ev[g, "Tp1"])
        if g >= 2:
            wait("act", "pe", ev[g - 2, "M2"])
        ev[g, "vt"] = emit("act", lambda e, o=T(p, "vtab"): e.copy(o[:, :], psT[:, :]), "act")

    def st_M2(g):
        p = g % 2
        wait("pe", "act", ev[g, "vt"])
        if g >= 1:
            wait("pe", "dve", ev[g - 1, "zs"])
        ev[g, "M2"] = emit("pe", lambda e, v=T(p, "vtab"): e.matmul(
            psF[:, :], S2, v[:, :], start=True, stop=True), "pe")

    def st_zs(g):
        p = g % 2
        wait("dve", "pe", ev[g, "M2"])
        if g >= 2:
            wait("dve", "pe", ev[g - 2, "M3"])
        ev[g, "zs"] = emit("dve", lambda e, o=T(p, "zsb"): e.tensor_tensor(
            o[:, :], psF[:, :], cD16, MUL), "dve")

    def st_M3(g):
        p = g % 2
        wait("pe", "dve", ev[g, "zs"])
        if g >= 1:
            wait("pe", "dve", ev[g - 1, "s"])
        emit("pe", lambda e, z=T(p, "zsb"): e.matmul(
            psW2[:, 0:512], S3, z[:, :], start=True, stop=True), "pe")
        ev[g, "M3"] = emit("pe", lambda e, z=T(p, "zsb"): e.matmul(
            psW2[:, 512:1024], S3sw, z[:, :], start=True, stop=True), "pe")

    def st_s(g):
        p = g % 2
        wait("dve", "pe", ev[g, "M3"])
        if g >= 2:
            wait("dve", "dve", ev[g - 2, "wp"])  # s2 WAR writeback
        ev[g, "s"] = emit("dve", lambda e, o=T(p, "s2"): e.tensor_tensor(
            o[:, :], psW2[:, :], cS, MUL), "dve")

    def st_wp(g):
        # on DVE: all-fp16 SBUF op -> 2x mode; self-wait for s(g) writeback
        p = g % 2
        wait("dve", "dve", ev[g, "s"])
        if g >= 2:
            wait("dve", "pe", ev[g - 2, "Tp2"])
        ev[g, "wp"] = emit("dve", lambda e, o=T(p, "wpb"), s=T(p, "s2"):
                           e.tensor_tensor(o[:, :], s[:, 0:512], s[:, 512:1024], ADD), "dve")

    def st_Tp2(g):
        # full-block transposes: psWt[:,128pr:+128] = wp-block^T; cols (pr,ri,g)
        p = g % 2
        wait("pe", "dve", ev[g, "wp"])
        if g >= 1:
            wait("pe", "act", ev[g - 1, "wt"])
        wpb = T(p, "wpb")
        for pr in range(4):
            ev[g, "Tp2"] = emit("pe", lambda e, w=wpb, pr=pr: e.matmul(
                psWt[:, 128 * pr:128 * pr + 128], w[:, 128 * pr:128 * pr + 128],
                I128, is_transpose=True, start=True, stop=True), "pe")

    def st_wt(g):
        p = g % 2
        wait("act", "pe", ev[g, "Tp2"])
        if g >= 2:
            wait("act", "pe", ev[g - 2, "M4"])
        ev[g, "wt"] = emit("act", lambda e, o=T(p, "wtab"): e.copy(
            o[:, :], psWt[:, :]), "act")

    def st_M4(g):
        p = g % 2
        wait("pe", "act", ev[g, "wt"])
        if g >= 1:
            wait("pe", "act", ev[g - 1, "outt"])
        # wtab cols (pr, ri, g): 4 strided-moving matmuls (one reload each)
        w = T(p, "wtab").rearrange("p (pr ri g) -> p pr ri g", ri=2, g=64)
        vr = w[:, :, 0, :]
        vi = w[:, :, 1, :]
        emit("pe", lambda e, v=vr: e.matmul(psO[:, 0:256], m7, v, start=True, stop=False), "pe")
        emit("pe", lambda e, v=vi: e.matmul(psO[:, 0:256], m8, v, start=False, stop=True), "pe")
        emit("pe", lambda e, v=vr: e.matmul(psO[:, 256:512], m9, v, start=True, stop=False), "pe")
        ev[g, "M4"] = emit("pe", lambda e, v=vi: e.matmul(psO[:, 256:512], m7, v, start=False, stop=True), "pe")

    def st_outt(g):
        p = g % 2
        wait("act", "pe", ev[g, "M4"])
        if g >= 2:
            wait("act", f"out{p}", ev[g - 2, "out"])
        pre = psO[:, 0:256].rearrange("p (pr g) -> p pr g", g=64)
        pim = psO[:, 256:512].rearrange("p (pr g) -> p pr g", g=64)
        emit("act", lambda e, o=T(p, "outt"), i=pre: e.copy(o[:, :, :, 0], i), "act")
        ev[g, "outt"] = emit("act", lambda e, o=T(p, "outt"), i=pim: e.copy(o[:, :, :, 1], i), "act")

    def st_out(g):
        p = g % 2
        gd = g % ngroups_data
        wait("sp", "act", ev[g, "outt"])
        def dout(e, gd=gd, outt=T(p, "outt")):
            og = out_ext[8 * gd: 8 * gd + 8, :, :, :].rearrange(
                "(pr h) eo g ri -> (h eo) pr (g ri)", h=2)
            return e.dma_start(out=og, in_=outt[:, :, :, :].rearrange(
                "p pr g ri -> p pr (g ri)"))
        ev[g, "out"] = emit("sp", dout, f"out{p}")

    chain = [st_in, st_A, st_M1, st_pb, st_t, st_q, st_Tp1, st_vt, st_M2,
             st_zs, st_M3, st_s, st_wp, st_Tp2, st_wt, st_M4, st_outt, st_out]
    enabled = set(chain if cut is None else chain[:cut])
    stages = [
        (0, st_in), (1, st_A),
        # DVE head: t, zs (no same-tick PE deps)
        (4, st_t), (8, st_zs),
        # PE order in tick: M3(->s), Tp2(->wt), Tp1(->vt), M4(->outt), M2, M1
        (9, st_M3), (11, st_Tp2), (6, st_Tp1), (12, st_M4), (7, st_M2), (2, st_M1),
        # DVE tail: wp, s
        (10, st_wp), (9, st_s),
        # ACT: pb, wt, vt, outt
        (3, st_pb), (11, st_wt), (6, st_vt), (12, st_outt),
        # pool: q
        (5, st_q),
        (13, st_out),
    ]
    for k in range(ngroups + 14):
        for d, fn in stages:
            g = k - d
            if 0 <= g < ngroups and fn in enabled:
                fn(g)

    for key in ("in0", "in1", "out0", "out1", "xb0", "xb1"):
        wait("sp", key, cn.c[key])

    with (
        nc.Block() as block,
        nc.semaphore("pe_sem") as pe_sem,
        nc.semaphore("dve_sem") as dve_sem,
        nc.semaphore("act_sem") as act_sem,
        nc.semaphore("pool_sem") as pool_sem,
        nc.semaphore("in0_sem") as in0_sem,
        nc.semaphore("in1_sem") as in1_sem,
        nc.semaphore("out0_sem") as out0_sem,
        nc.semaphore("out1_sem") as out1_sem,
        nc.semaphore("cdma_sem") as cdma_sem,
        nc.semaphore("xb0_sem") as xb0_sem,
        nc.semaphore("xb1_sem") as xb1_sem,
    ):
        sems = {"pe": pe_sem, "dve": dve_sem, "act": act_sem, "pool": pool_sem,
                "in0": in0_sem, "in1": in1_sem, "out0": out0_sem,
                "out1": out1_sem, "cdma": cdma_sem,
                "xb0": xb0_sem, "xb1": xb1_sem}

        def runner(eng_name):
            def body(e):
                for item in prog[eng_name]:
                    if item[0] == "wait":
                        e.wait_ge(sems[item[1]], item[2])
                    else:
                        _, fn, sem_key = item
                        inst = fn(e)
                        inst.then_inc(sems[sem_key], 16 if sem_key in DMA_KEYS else 1)
            return body

        block.sync(runner("sp"))
        block.tensor(runner("pe"))
        block.vector(runner("dve"))
        block.scalar(runner("act"))
        block.gpsimd(runner("pool"))

    ctx.close()
    return nc


_NC_CACHE = {}


def kernel(x, A, D):
    from concourse.bass_utils import run_bass_kernel_spmd
    x = np.asarray(x, dtype=np.float32)
    B = x.shape[0]
    R = B // 8
    if R not in _NC_CACHE:
        _NC_CACHE[R] = build_nc_v4(R)
    nc = _NC_CACHE[R]
    mats = make_mats2()
    coefs = make_coefs2(A, D)
    xs = x.reshape(8, R, 64, 64, 2)
    in_maps = [
        {"x": np.ascontiguousarray(xs[i]), "cmats_d": mats, "ccoef_d": coefs}
        for i in range(8)
    ]
    res = run_bass_kernel_spmd(nc, in_maps, core_ids=list(range(8)))
    out = np.concatenate(
        [res.results[i]["out"].reshape(R, 4096, 2) for i in range(8)], axis=0
    )
    return out
